# revision 5
# baseline (speedup 1.0000x reference)
"""Canny edge filter on 8 Trainium2 NeuronCores (Bass/Tile) — v3.

v3 vs v2:
  - uint8 device output (host casts to f32): 4x less output HBM traffic.
  - Row shifts act on g2 directly (f32r bitcast): 4 shift matmuls/strip
    instead of 8 (PE 34 -> 30 matmuls/strip).
  - No validity-mask multiply (g2m dropped): the per-strip max reduce is
    fused into the nms multiply via tensor_tensor_reduce on the valid
    partition slice only.
  - Final threshold combine (mult+add) moved to Pool; u8 output tile is a
    bitcast view of an f32 work tile (no extra SBUF).

Works in the scale-invariant squared domain (no sqrt / atan2 / divide).
"""
import sys

sys.path.insert(0, "/opt/trn_rl_repo")

import numpy as np

import concourse.bass as bass
import concourse.mybir as mybir
import concourse.tile as tile
from concourse.bass_utils import run_bass_kernel_spmd
from concourse.vector_clock import ScopedClock

f32 = np.float32
dt = mybir.dt
Alu = mybir.AluOpType
Act = mybir.ActivationFunctionType

N_CORES = 8
H = W = 1024
P = 128
ROW_STARTS = [0, 120, 240, 360, 480, 600, 720, 840, 896]
VALID = [(0, 124)] + [(4, 124)] * 7 + [(68, 128)]
CTILES = [(0, 512), (512, 512)]                   # conv output col tiles
T22 = float(np.float64(np.tan(np.deg2rad(22.5))))
USE_TTR = False  # walrus here cannot codegen InstTensorTensorReduce ('ISA wrong length')
HI2 = float(np.float32(0.15) * np.float32(0.15))
LO2 = float(np.float32(0.05) * np.float32(0.05))

# ---------------------------------------------------------------- tile patch


def _patched_drain_and_barrier(self, tick_clock, wait_clock):
    nc = self.nc
    drain_inst = nc.sync.drain()
    wait_clock.add_sem_waits(
        drain_inst.ins, ScopedClock({None: tick_clock.global_clock})
    )
    si = drain_inst.ins.sync_info
    waits = list(si.on_wait or []) if si is not None else []
    if len(waits) > 1:
        si.on_wait = waits[:1]
        drain_inst.ins.sync_info = si
        by_id = dict(self.sems.allocated())
        for w in waits[1:]:
            h = by_id.get(w.id)
            if h is None:
                for hh in by_id.values():
                    if getattr(hh, "name", None) == w.ant_name:
                        h = hh
                        break
            assert h is not None, f"cannot resolve semaphore for wait {w}"
            nc.sync.wait_ge(h, w.wait_value)
    nc.all_engine_barrier()
    popped = nc._tile_sem_poison_stack.pop()
    assert popped is self._sem_poison
    nc.clear_and_free_semaphores(list(self.sems.allocated().values()))
    nc.all_engine_barrier()


def _split_excess_waits(nc):
    n_split = 0
    for fn in nc.m.functions:
        for bb in fn.blocks:
            insts = list(bb.instructions)
            out = []
            changed = False
            for ins in insts:
                si = ins.sync_info
                waits = list(si.on_wait) if si is not None and si.on_wait else []
                cap = 2 if isinstance(ins, mybir.InstEventSemaphore) else 1
                if len(waits) > cap:
                    changed = True
                    n_split += 1
                    excess = waits[:-cap]
                    for ci, w in enumerate(excess):
                        nop = mybir.InstNoOp(
                            name=f"{ins.name}-wsplit-{ci}", ins=[], outs=[]
                        )
                        nop.debug = ins.debug
                        nop.engine = ins.engine
                        nop.sync_info = mybir.SyncInfo(on_wait=[w], on_update=[])
                        nc.register_instruction(nop, overwrite=True)
                        out.append(nop)
                    si.on_wait = waits[-cap:]
                    ins.sync_info = si
                out.append(ins)
            if changed:
                bb.instructions = out
    return n_split


tile.TileContext._drain_and_barrier = _patched_drain_and_barrier

# ------------------------------------------------------------- band matrices


def _gauss_k1():
    ax = np.arange(5, dtype=np.float64) - 2.0
    k1 = np.exp(-(ax**2) / 2.0)
    k1 = k1 / k1.sum()
    return k1.astype(f32).astype(np.float64)


def _refl(k, variant):
    if 0 <= k < P:
        return k
    if k < 0 and variant == 0:
        return -k
    if k > P - 1 and variant == 2:
        return 2 * (P - 1) - k
    return None


def _band(taps, variant):
    m = np.zeros((P, P), dtype=np.float64)
    for i in range(P):
        for a, c in taps.items():
            k = _refl(i + a, variant)
            if k is not None:
                m[k, i] += c
    return m


def _comp_h(staps, k1):
    h = np.zeros(7, dtype=np.float64)
    for s, cf in staps.items():
        for t in range(-2, 3):
            h[s + t + 3] += cf * k1[t + 2]
    return h


def make_consts():
    k1 = _gauss_k1()
    gtaps = {a - 2: k1[a] for a in range(5)}
    hx = _comp_h({-1: -1.0, 1: 1.0}, k1)
    hy = _comp_h({-1: 1.0, 0: 2.0, 1: 1.0}, k1)
    consts = {}
    for v in range(3):
        mg = _band(gtaps, v)
        cx = mg @ _band({-1: 1.0, 0: 2.0, 1: 1.0}, v)   # ix vertical
        cy = mg @ _band({-1: 1.0, 1: -1.0}, v)          # iy vertical
        for j in range(7):
            if j != 3:
                consts[f"cx{j}_{v}"] = (cx * hx[j]).astype(f32)
            consts[f"cy{j}_{v}"] = (cy * hy[j]).astype(f32)
    up = np.zeros((P, P), dtype=f32)  # gU[i] = g2[i-1]
    dn = np.zeros((P, P), dtype=f32)  # gD[i] = g2[i+1]
    for i in range(P):
        if i >= 1:
            up[i - 1, i] = 1.0
        if i <= P - 2:
            dn[i + 1, i] = 1.0
    consts["up"] = up
    consts["dn"] = dn
    consts["ones1"] = np.ones((1, P), dtype=f32)
    consts["id128"] = np.eye(P, dtype=f32)
    # conv-valid row windows per variant; folded into the Square scale so
    # garbage conv rows contribute 0 to g2 (and so to the image max)
    for v, (a, b) in enumerate([(0, 125), (3, 125), (67, 128)]):
        vm = np.zeros((P, 1), dtype=f32)
        vm[a:b] = 1.0
        consts[f"vm_{v}"] = vm
    return consts


# ------------------------------------------------------------ kernel builder


def build_program(n_img, reps=1):
    consts = make_consts()
    nc = bass.Bass("TRN2", target_bir_lowering=False, debug=False)
    # input is host-pre-padded with the 4-column reflect halo
    x_d = nc.dram_tensor(
        "x", [n_img, H, W + 8], dt.float32r, kind="ExternalInput"
    ).ap()
    o_d = nc.dram_tensor("out", [n_img, H, W], dt.float32, kind="ExternalOutput").ap()

    def _cdt(k):
        if k in ("ones1", "id128") or k.startswith("vm_"):
            return dt.float32
        return dt.float32r

    c_d = {
        k: nc.dram_tensor(k, list(v.shape), _cdt(k), kind="ExternalInput").ap()
        for k, v in consts.items()
    }

    with tile.TileContext(nc) as tc:
        with (
            tc.tile_pool(name="cpool", bufs=1) as cpool,
            tc.tile_pool(name="xin", bufs=2) as xin,
            tc.tile_pool(name="work", bufs=2) as work,
            tc.tile_pool(name="nmsbuf", bufs=2) as nmsbuf,
            tc.tile_pool(name="small", bufs=2) as small,
            tc.tile_pool(name="p_cv", bufs=2, space="PSUM") as p_cv,
            tc.tile_pool(name="p_ud", bufs=2, space="PSUM") as p_ud,
        ):
            cts = {}
            for k, v in consts.items():
                t = cpool.tile(list(v.shape), _cdt(k), tag=f"c_{k}", name=f"c_{k}")
                nc.sync.dma_start(t[:], c_d[k][:])
                cts[k] = t

            # edge zero-columns are rewritten identically every buffer
            # rotation; emitting them for the first `bufs` instances of a
            # tag suffices (the pool ring reuses the same SBUF bytes).
            _msets = {}

            def memset_once(tag, ap):
                n = _msets.get(tag, 0)
                if n < 2:
                    _msets[tag] = n + 1
                    nc.gpsimd.memset(ap, 0.0)

            # ---- software-pipelined strips: front(s) runs the input DMA +
            # composite convolutions; back(s) runs shifts + NMS.
            def emit_front(img, key, st):
                s = key[1]
                r0 = ROW_STARTS[s]
                v = 0 if s == 0 else (2 if s == 8 else 1)
                xt = xin.tile([P, W + 8], dt.float32r, tag="xt")
                nc.sync.dma_start(xt[:], x_d[img, r0 : r0 + P, :])
                gx = work.tile([P, W], dt.float32r, tag="gx")
                gy = work.tile([P, W], dt.float32r, tag="gy")
                sm = work.tile([P, W], dt.float32, tag="sm")
                siy = work.tile([P, W], dt.float32, tag="siy")
                for c0, wt in CTILES:
                    ixp = p_cv.tile([P, 512], dt.float32, tag="ixp")
                    taps = [0, 1, 2, 4, 5, 6]
                    for bi, j in enumerate(taps):
                        nc.tensor.matmul(
                            ixp[:, 0:wt],
                            cts[f"cx{j}_{v}"][:],
                            xt[:, c0 + j + 1 : c0 + j + 1 + wt],
                            start=(bi == 0),
                            stop=(bi == len(taps) - 1),
                        )
                    iyp = p_cv.tile([P, 512], dt.float32, tag="iyp")
                    for j in range(7):
                        nc.tensor.matmul(
                            iyp[:, 0:wt],
                            cts[f"cy{j}_{v}"][:],
                            xt[:, c0 + j + 1 : c0 + j + 1 + wt],
                            start=(j == 0),
                            stop=(j == 6),
                        )
                    nc.scalar.activation(
                        gx[:, c0 : c0 + wt], ixp[:, 0:wt], Act.Square,
                        scale=cts[f"vm_{v}"][:],
                    )
                    nc.scalar.activation(
                        gy[:, c0 : c0 + wt], iyp[:, 0:wt], Act.Square,
                        scale=cts[f"vm_{v}"][:],
                    )
                    # sm = sign(ix)*sign(iy): same sign as ix*iy (zero ties
                    # are claimed by the 0/90 bins anyway).  Product split
                    # Pool/DVE per ctile for engine balance.
                    nc.scalar.activation(
                        sm[:, c0 : c0 + wt], ixp[:, 0:wt], Act.Sign
                    )
                    nc.scalar.activation(
                        siy[:, c0 : c0 + wt], iyp[:, 0:wt], Act.Sign
                    )
                    nc.gpsimd.tensor_tensor(
                        sm[:, c0 : c0 + wt], sm[:, c0 : c0 + wt],
                        siy[:, c0 : c0 + wt], Alu.mult,
                    )
                st[key] = (gx, gy, sm)

            def emit_back_a(img, key, st):
                s = key[1]
                gx, gy, sm = st.pop(key)
                gxf = gx[:].bitcast(dt.float32)
                gyf = gy[:].bitcast(dt.float32)
                g2s = work.tile([P, W + 2], dt.float32, tag="g2s")
                g2f = g2s[:]
                nc.gpsimd.tensor_tensor(
                    g2s[:, 1 : W + 1], gxf, gyf, Alu.add
                )
                nc.gpsimd.memset(g2s[:, 0:1], 0.0)
                nc.gpsimd.memset(g2s[:, W + 1 : W + 2], 0.0)
                # direction masks ({0,1} exact: predicates for copy_predicated)
                c0m = work.tile([P, W], dt.float32, tag="c0m")
                nc.vector.scalar_tensor_tensor(
                    c0m[:], gxf, T22 * T22, gyf, Alu.mult, Alu.is_gt
                )
                c90m = work.tile([P, W], dt.float32, tag="keep")
                nc.vector.scalar_tensor_tensor(
                    c90m[:], gyf, T22 * T22, gxf, Alu.mult, Alu.is_ge
                )
                # diag predicate: sm = sign(ix)*sign(iy) in {-1,0,1};
                # sm+1 in {0,1,2} is zero iff sm==-1.  sm==0 pixels are always
                # overridden by c0m/c90m (ix==0 -> c90m, iy==0 -> c0m).
                nc.gpsimd.tensor_scalar(
                    sm[:], sm[:], 1.0, 1.0, Alu.add, Alu.mult
                )
                m0 = work.tile([P, W], dt.float32, tag="m0")
                nc.vector.tensor_tensor(
                    m0[:], g2s[:, 0:W], g2s[:, 2 : W + 2], Alu.max
                )
                gus = work.tile([P, W + 2], dt.float32, tag="gus")
                gds = work.tile([P, W + 2], dt.float32, tag="gds")
                nc.gpsimd.memset(gus[:, 0:1], 0.0)
                nc.gpsimd.memset(gus[:, W + 1 : W + 2], 0.0)
                nc.gpsimd.memset(gds[:, 0:1], 0.0)
                nc.gpsimd.memset(gds[:, W + 1 : W + 2], 0.0)
                for c0, wt in CTILES:
                    gup = p_ud.tile([P, 512], dt.float32, tag="gup")
                    gdp = p_ud.tile([P, 512], dt.float32, tag="gdp")
                    nc.tensor.matmul(
                        gup[:, 0:wt], cts["up"][:], gx[:, c0 : c0 + wt],
                        start=True, stop=False,
                    )
                    nc.tensor.matmul(
                        gup[:, 0:wt], cts["up"][:], gy[:, c0 : c0 + wt],
                        start=False, stop=True,
                    )
                    nc.tensor.matmul(
                        gdp[:, 0:wt], cts["dn"][:], gx[:, c0 : c0 + wt],
                        start=True, stop=False,
                    )
                    nc.tensor.matmul(
                        gdp[:, 0:wt], cts["dn"][:], gy[:, c0 : c0 + wt],
                        start=False, stop=True,
                    )
                    nc.scalar.copy(
                        gus[:, 1 + c0 : 1 + c0 + wt], gup[:, 0:wt]
                    )
                    nc.scalar.copy(
                        gds[:, 1 + c0 : 1 + c0 + wt], gdp[:, 0:wt]
                    )
                return (g2s, c0m, c90m, sm, m0, gus, gds)

            def emit_back_b(key, bst, nmsb, pm):
                s = key[1]
                g2s, c0m, c90m, sm, m0, gus, gds = bst
                g2f = g2s[:]
                m90 = work.tile([P, W], dt.float32, tag="m90")
                m45 = work.tile([P, W], dt.float32, tag="m45")
                M = work.tile([P, W], dt.float32, tag="M")
                nc.vector.tensor_tensor(
                    m90[:], gus[:, 1 : W + 1], gds[:, 1 : W + 1], Alu.max
                )
                nc.vector.tensor_tensor(
                    m45[:], gus[:, 2 : W + 2], gds[:, 0:W], Alu.max
                )
                nc.vector.tensor_tensor(
                    M[:], gus[:, 0:W], gds[:, 2 : W + 2], Alu.max
                )
                nc.vector.copy_predicated(M[:], sm[:].bitcast(dt.int32), m45[:])
                nc.vector.copy_predicated(M[:], c90m[:].bitcast(dt.int32), m90[:])
                nc.vector.copy_predicated(M[:], c0m[:].bitcast(dt.int32), m0[:])
                keep = work.tile([P, W], dt.float32, tag="m90")
                nc.vector.tensor_tensor(
                    keep[:], g2f[:, 1 : W + 1], M[:], Alu.is_ge
                )
                # nms + per-strip max.  Full tile: conv-garbage rows have
                # g2 == 0 (vm-scaled squares); boundary rows are bounded by
                # a true g2 value, so the max is never polluted.
                if USE_TTR:
                    nc.vector.tensor_tensor_reduce(
                        nmsb[s][:],
                        g2f[:, 1 : W + 1],
                        keep[:],
                        1.0,
                        0.0,
                        Alu.mult,
                        Alu.max,
                        pm[:, s : s + 1],
                    )
                else:
                    nc.vector.tensor_tensor(
                        nmsb[s][:], g2f[:, 1 : W + 1], keep[:], Alu.mult
                    )

                def _reduce():
                    nc.vector.tensor_reduce(
                        pm[:, s : s + 1], nmsb[s][:], mybir.AxisListType.X, Alu.max
                    )
                return _reduce

            imgs = [i for _r in range(reps) for i in range(n_img)]
            img_state = {}

            def start_image(idx):
                nmsb = [
                    nmsbuf.tile([P, W], dt.float32, tag=f"nmsb{s}", name=f"nmsb{s}")
                    for s in range(9)
                ]
                pm = small.tile([P, 16], dt.float32, tag="pm")
                nc.vector.memset(pm[:], 0.0)
                img_state[idx] = {"nmsb": nmsb, "pm": pm}

            def emit_epilogue(idx):
                ist = img_state[idx]
                pm = ist["pm"]
                pm1 = small.tile([P, 1], dt.float32, tag="pm1")
                nc.vector.tensor_reduce(pm1[:], pm[:], mybir.AxisListType.X, Alu.max)
                pmt = p_ud.tile([P, 512], dt.float32, tag="gup")
                nc.tensor.transpose(pmt[0:1, 0:P], pm1[:], cts["id128"][:])
                pmc = small.tile([1, 1], dt.float32, tag="pmc")
                nc.vector.tensor_reduce(
                    pmc[0:1, :], pmt[0:1, 0:P], mybir.AxisListType.X, Alu.max
                )
                pbc = p_ud.tile([P, 512], dt.float32, tag="gdp")
                nc.tensor.matmul(
                    pbc[:, 0:1], cts["ones1"][:], pmc[0:1, :], start=True, stop=True
                )
                hi2n = small.tile([P, 1], dt.float32, tag="hi2n")
                nc.scalar.activation(hi2n[:], pbc[:, 0:1], Act.Copy, scale=-HI2)
                lo2n = small.tile([P, 1], dt.float32, tag="lo2n")
                nc.scalar.activation(lo2n[:], pbc[:, 0:1], Act.Copy, scale=-LO2)
                ist["hi2n"] = hi2n
                ist["lo2n"] = lo2n

            def pass2_strip(idx, s):
                # out = 115*sign(nms-hi) + 12.5*sign(nms-lo) + 127.5
                #     -> {0, 25, 255}; all constants exact in f32
                ist = img_state[idx]
                img = imgs[idx]
                nmsb = ist["nmsb"]
                p0, p1 = VALID[s]
                r0 = ROW_STARTS[s]
                shi = work.tile([P, W], dt.float32, tag="gx")
                nc.scalar.activation(
                    shi[:], nmsb[s][:], Act.Sign, bias=ist["hi2n"][:]
                )
                slo = work.tile([P, W], dt.float32, tag="gy")
                nc.scalar.activation(
                    slo[:], nmsb[s][:], Act.Sign, bias=ist["lo2n"][:]
                )
                t1 = work.tile([P, W], dt.float32, tag="keep")
                nc.gpsimd.tensor_scalar(
                    t1[:], shi[:], 115.0, 127.5, Alu.mult, Alu.add
                )
                ot = work.tile([P, W], dt.float32, tag="sm")
                nc.vector.scalar_tensor_tensor(
                    ot[:], slo[:], 12.5, t1[:], Alu.mult, Alu.add
                )
                nc.sync.dma_start(o_d[img, r0 + p0 : r0 + p1, :], ot[p0:p1, :])
                if s == 8:
                    del img_state[idx]

            seq = [(idx, s) for idx in range(len(imgs)) for s in range(9)]
            st = {}
            pending = []  # deferred per-strip reduces
            p2q = []      # pass-2 strips ready to emit

            def flush_reduce():
                if pending:
                    pending.pop(0)()

            for k, (idx, s) in enumerate(seq):
                bst = None
                if k >= 1:
                    pidx, ps = seq[k - 1]
                    bst = emit_back_a(imgs[pidx], (pidx, ps), st)
                if s == 0:
                    start_image(idx)
                emit_front(imgs[idx], (idx, s), st)
                if len(pending) > 1:
                    flush_reduce()
                # spread pass-2 one strip per step, skipping each image's
                # first step (epilogue just issued); catch up on the last
                if p2q and s != 0:
                    pass2_strip(*p2q.pop(0))
                    if s == 8 and p2q:
                        pass2_strip(*p2q.pop(0))
                if bst is not None:
                    pidx, ps = seq[k - 1]
                    pending.append(emit_back_b(
                        (pidx, ps), bst,
                        img_state[pidx]["nmsb"], img_state[pidx]["pm"],
                    ))
                    if ps == 8:
                        while pending:
                            flush_reduce()
                        emit_epilogue(pidx)
                        p2q.extend((pidx, s2) for s2 in range(9))
            pidx, ps = seq[-1]
            bst = emit_back_a(imgs[pidx], (pidx, ps), st)
            pending.append(emit_back_b(
                (pidx, ps), bst,
                img_state[pidx]["nmsb"], img_state[pidx]["pm"],
            ))
            while pending:
                flush_reduce()
            emit_epilogue(pidx)
            p2q.extend((pidx, s2) for s2 in range(9))
            while p2q:
                pass2_strip(*p2q.pop(0))

    _split_excess_waits(nc)
    return nc, consts


_CACHE = {}


def _get_program(n_img, reps=1):
    key = (n_img, reps, USE_TTR)
    if key not in _CACHE:
        _CACHE[key] = build_program(n_img, reps)
    return _CACHE[key]


def prep_input(x):
    """[B,1,H,W] -> per-core [n_img,H,W+8] with 4-col reflect halo."""
    B = x.shape[0]
    n_img = B // N_CORES
    xr = x.reshape(B, H, W).astype(np.float32)
    xp = np.pad(xr, ((0, 0), (0, 0), (4, 4)), mode="reflect")
    return np.ascontiguousarray(xp.reshape(N_CORES, n_img, H, W + 8))


def postprocess_output(out):
    return out


def kernel(x: np.ndarray) -> np.ndarray:
    B = x.shape[0]
    n_img = B // N_CORES
    nc, consts = _get_program(n_img)
    xs = prep_input(x)
    in_maps = []
    for c in range(N_CORES):
        m = {"x": xs[c]}
        m.update(consts)
        in_maps.append(m)
    res = run_bass_kernel_spmd(nc, in_maps, core_ids=list(range(N_CORES)))
    out = np.stack([res.results[c]["out"] for c in range(N_CORES)], axis=0)
    return postprocess_output(out).reshape(B, 1, H, W)
